# revision 13
# baseline (speedup 1.0000x reference)
"""Trainium2 Bass kernel for ConvolutionFeatureProcessor.

Math (per item, matching the jax reference):
  h[t]   = relu(b1 + sum_k x[t+k] @ w1k^T)          t in [0, T-2)
  pooled = (1/(L-2)) * sum_{t<L-2} h[t]             (masked mean)
  p2     = W2 @ pooled + b2      (td2 linear -> commutes with the mean)
  out    = MLP(p2)               (64 -> 256 -> 256 -> 512)

Strategy (8 cores, data parallel over the batch, ragged-aware):
  - Host sorts items by length and packs 2 similar-length items per
    128-partition "slot"; slot s processes only ceil(Lmax_s/512) chunks
    of 512 frames (compile-time constants -> program cache key).
  - Host pre-transposes x to [d, t] layout, zero-pads each item's tail,
    and casts to bf16, so the device does a single contiguous bf16 load
    per slot (half the HBM traffic, no on-device transpose).
  - No masking on device: tails are zero-padded, and the host computes
    the exact correction (2 boundary frames + count * relu(b1)) that the
    device subtracts from the pooled sum.
  - Conv: weight-stationary k-sweeps (3 matmuls per chunk, block-diag
    w_k^T lhsT), one PSUM bank per chunk (up to 8 in flight).
  - relu+bias+pool-accum in ONE tensor_scalar op per chunk, alternating
    between the Vector and GpSimd engines.
  - td2 is folded into the first MLP layer on host; MLP runs in bf16
    feature-major; output is written [feat, item] and untangled on host.
"""

import numpy as np
import ml_dtypes

B, T, D, OUT = 128, 4096, 64, 512
NCORES = 8
BS = B // NCORES       # items per core
NSLOT = BS // 2        # pair-slots per core
TC = 512               # conv chunk (free dim per matmul / psum bank)

_CACHE = {}


def _build(ns_list, dbg=False):
    """Compile the SPMD program for per-slot chunk counts `ns_list`."""
    import concourse.bacc as bacc
    import concourse.mybir as mybir
    import concourse.tile as tile

    f32 = mybir.dt.float32
    bf16 = mybir.dt.bfloat16
    fp8 = mybir.dt.float8e4
    AX = mybir.AxisListType
    OP = mybir.AluOpType
    AF = mybir.ActivationFunctionType

    F_tot = sum(TC * n for n in ns_list)

    nc = bacc.Bacc("TRN2", target_bir_lowering=False, debug=False)

    xts_d = [nc.dram_tensor(f"xt{s}", [128, TC * ns_list[s] + 2], fp8,
                             kind="ExternalInput").ap()
             for s in range(NSLOT)]
    wpk_d = nc.dram_tensor("wpk", [128, 3 * 128], fp8, kind="ExternalInput").ap()
    bpk_d = nc.dram_tensor("bpk", [128, 1], f32, kind="ExternalInput").ap()
    inv_d = nc.dram_tensor("inv", [128, NSLOT], f32, kind="ExternalInput").ap()
    corr_d = nc.dram_tensor("corr", [64, BS], f32, kind="ExternalInput").ap()
    wl1_d = nc.dram_tensor("wl1", [64, 256], bf16, kind="ExternalInput").ap()
    wl2_d = nc.dram_tensor("wl2", [128, 512], bf16, kind="ExternalInput").ap()
    wl3_d = nc.dram_tensor("wl3", [128, 1024], bf16, kind="ExternalInput").ap()
    bl1_d = nc.dram_tensor("bl1", [128, 2], f32, kind="ExternalInput").ap()
    bl2_d = nc.dram_tensor("bl2", [128, 2], f32, kind="ExternalInput").ap()
    bl3_d = nc.dram_tensor("bl3", [128, 4], f32, kind="ExternalInput").ap()
    y_d = nc.dram_tensor("y", [128, 4 * BS], f32, kind="ExternalOutput").ap()
    if dbg:
        dbg_pool = nc.dram_tensor("dbg_pool", [128, NSLOT], f32,
                                  kind="ExternalOutput").ap()
        dbg_part = nc.dram_tensor("dbg_part", [128, 8], f32,
                                  kind="ExternalOutput").ap()
        dbg_h = nc.dram_tensor("dbg_h", [128, TC], f32,
                               kind="ExternalOutput").ap()
        dbg_pl = nc.dram_tensor("dbg_pl", [64, BS], f32,
                                kind="ExternalOutput").ap()

    with tile.TileContext(nc) as tc:
        with (
            tc.tile_pool(name="const", bufs=1) as const,
            tc.tile_pool(name="xt", bufs=3) as xt_pool,
            tc.tile_pool(name="junk", bufs=1) as junk_pool,
            tc.tile_pool(name="smalls", bufs=2) as smalls,
            tc.tile_pool(name="ps", bufs=2, space="PSUM") as ps,
        ):
            # ---------------- constant loads (pure DMA) ----------------
            W_pack = const.tile([128, 3 * 128], fp8, tag="wpk")
            nc.sync.dma_start(out=W_pack[:], in_=wpk_d[:])
            b_pack = const.tile([128, 1], f32, tag="bpk")
            nc.sync.dma_start(out=b_pack[:], in_=bpk_d[:])
            inv_all = const.tile([128, NSLOT], f32, tag="inv")
            nc.sync.dma_start(out=inv_all[:], in_=inv_d[:])
            corr_sb = const.tile([64, BS], f32, tag="corr")
            nc.sync.dma_start(out=corr_sb[:], in_=corr_d[:])
            Wl1T = const.tile([64, 256], bf16, tag="wl1")
            nc.scalar.dma_start(out=Wl1T[:], in_=wl1_d[:])
            Wl2T = const.tile([128, 512], bf16, tag="wl2")
            nc.scalar.dma_start(out=Wl2T[:], in_=wl2_d[:])
            Wl3T = const.tile([128, 1024], bf16, tag="wl3")
            nc.scalar.dma_start(out=Wl3T[:], in_=wl3_d[:])
            bl1_sb = const.tile([128, 2], f32, tag="bl1")
            nc.scalar.dma_start(out=bl1_sb[:], in_=bl1_d[:])
            bl2_sb = const.tile([128, 2], f32, tag="bl2")
            nc.scalar.dma_start(out=bl2_sb[:], in_=bl2_d[:])
            bl3_sb = const.tile([128, 4], f32, tag="bl3")
            nc.scalar.dma_start(out=bl3_sb[:], in_=bl3_d[:])

            pooled_all = const.tile([128, NSLOT], f32, tag="pooled_all")
            junk_v = junk_pool.tile([128, 4 * TC], bf16, tag="junk_v")
            junk_g = junk_pool.tile([128, 4 * TC], bf16, tag="junk_g")
            zeros_sb = const.tile([128, 4 * TC], bf16, tag="zeros_sb")
            nc.vector.memset(zeros_sb[:], 0.0)

            # ---------------- per-slot streaming conv ----------------
            gctr = 0  # global accum-group counter for DVE/ACT alternation
            for si, s in enumerate(sorted(range(NSLOT),
                                          key=lambda t: ns_list[t])):
                n_s = ns_list[s]
                F_s = TC * n_s
                xt = xt_pool.tile([128, F_s + 2], fp8, name=f"xt{s}", tag="xt")
                ldq = nc.gpsimd if si % 2 == 0 else nc.sync
                ldq.dma_start(out=xt[:], in_=xts_d[s][:])

                ngrp = -(-n_s // 4)
                grp_n = [min(4, n_s - 4 * q) for q in range(ngrp)]
                psg = [ps.tile([128, 4 * TC], f32, name=f"ps{s}_{q}", tag="ps")
                       for q in range(ngrp)]
                for k in range(3):
                    lhsT = W_pack[:, 128 * k:128 * (k + 1)]
                    for n in range(n_s):
                        N = TC if n < n_s - 1 else TC - 2
                        q, r = divmod(n, 4)
                        nc.tensor.matmul(
                            out=psg[q][:, r * TC:r * TC + N], lhsT=lhsT,
                            rhs=xt[:, TC * n + k:TC * n + k + N],
                            start=(k == 0), stop=(k == 2))

                partials = smalls.tile([128, ngrp], f32, name=f"pt{s}",
                                       tag="partials")
                for q in range(ngrp):
                    W = (grp_n[q] - 1) * TC + \
                        (TC if 4 * q + grp_n[q] < n_s else TC - 2)
                    if gctr % 2 == 0:
                        # DVE: out = (in + bias) max 0; accum_out = sum(out)
                        nc.vector.scalar_tensor_tensor(
                            out=junk_v[:, :W], in0=psg[q][:, :W],
                            scalar=b_pack[:], in1=zeros_sb[:, :W],
                            op0=OP.add, op1=OP.max,
                            accum_out=partials[:, q:q + 1])
                    else:
                        # scalar engine: out/accum = relu(in + bias), sum
                        nc.scalar.activation(
                            out=junk_g[:, :W], in_=psg[q][:, :W],
                            func=AF.Relu, bias=b_pack[:],
                            accum_out=partials[:, q:q + 1])
                    gctr += 1
                    if dbg and s == 0 and q == 0:
                        hcp = smalls.tile([128, TC], f32, tag="hcp")
                        jsrc = junk_v if (gctr - 1) % 2 == 0 else junk_g
                        nc.vector.tensor_copy(hcp[:], jsrc[:, :TC])
                        nc.sync.dma_start(out=dbg_h[:], in_=hcp[:])

                pool_sum = smalls.tile([128, 1], f32, name=f"psum{s}",
                                       tag="pool_sum")
                if ngrp > 1:
                    nc.vector.tensor_reduce(out=pool_sum[:], in_=partials[:],
                                            axis=AX.X, op=OP.add)
                else:
                    nc.vector.tensor_copy(pool_sum[:], partials[:])
                nc.vector.tensor_scalar(
                    out=pooled_all[:, s:s + 1], in0=pool_sum[:],
                    scalar1=inv_all[:, s:s + 1], scalar2=None, op0=OP.mult)
                if dbg and s == 0:
                    nc.sync.dma_start(out=dbg_part[:, :n_s], in_=partials[:])

            # ---------------- pooled -> MLP (bf16) ----------------
            # PL cols: item j = 2s+pos; even cols from partitions 0:64,
            # odd cols from partitions 64:128 (partition move via DMA).
            PLf = const.tile([64, BS], f32, tag="PLf")
            pv = pooled_all[:].rearrange("p (s one) -> p s one", one=1)
            plv = PLf[:].rearrange("p (s two) -> p s two", two=2)
            nc.vector.tensor_copy(plv[:, :, 0:1], pv[0:64])
            nc.sync.dma_start(out=plv[:, :, 1:2], in_=pv[64:128])
            # subtract host correction; cast to bf16
            PL = const.tile([64, BS], bf16, tag="PL")
            nc.vector.tensor_tensor(out=PL[:], in0=PLf[:], in1=corr_sb[:],
                                    op=OP.subtract)
            if dbg:
                nc.sync.dma_start(out=dbg_pool[:], in_=pooled_all[:])
                PLc = const.tile([64, BS], f32, tag="PLc")
                nc.vector.tensor_copy(PLc[:], PL[:])
                nc.sync.dma_start(out=dbg_pl[:], in_=PLc[:])

            z1 = const.tile([128, 2 * BS], bf16, tag="z1")
            for m in range(2):
                pz = ps.tile([128, 4 * TC], f32, name=f"pz1_{m}", tag="ps")[:, :BS]
                nc.tensor.matmul(out=pz[:], lhsT=Wl1T[:, m * 128:(m + 1) * 128],
                                 rhs=PL[:], start=True, stop=True)
                nc.scalar.activation(out=z1[:, m * BS:(m + 1) * BS], in_=pz[:],
                                     func=AF.Relu, bias=bl1_sb[:, m:m + 1])
            z2 = const.tile([128, 2 * BS], bf16, tag="z2")
            for m in range(2):
                pz = ps.tile([128, 4 * TC], f32, name=f"pz2_{m}", tag="ps")[:, :BS]
                for kc in range(2):
                    nc.tensor.matmul(
                        out=pz[:],
                        lhsT=Wl2T[:, 256 * kc + 128 * m:256 * kc + 128 * (m + 1)],
                        rhs=z1[:, kc * BS:(kc + 1) * BS],
                        start=(kc == 0), stop=(kc == 1))
                nc.scalar.activation(out=z2[:, m * BS:(m + 1) * BS], in_=pz[:],
                                     func=AF.Relu, bias=bl2_sb[:, m:m + 1])
            y_sb = const.tile([128, 4 * BS], f32, tag="y_sb")
            for m in range(4):
                pz = ps.tile([128, 4 * TC], f32, name=f"pz3_{m}", tag="ps")[:, :BS]
                for kc in range(2):
                    nc.tensor.matmul(
                        out=pz[:],
                        lhsT=Wl3T[:, 512 * kc + 128 * m:512 * kc + 128 * (m + 1)],
                        rhs=z2[:, kc * BS:(kc + 1) * BS],
                        start=(kc == 0), stop=(kc == 1))
                nc.scalar.activation(out=y_sb[:, m * BS:(m + 1) * BS], in_=pz[:],
                                     func=AF.Identity, bias=bl3_sb[:, m:m + 1])
            nc.sync.dma_start(out=y_d[:], in_=y_sb[:])

    nc.compile()
    return nc


def _get_nc(ns_key):
    nc = _CACHE.get(ns_key)
    if nc is None:
        nc = _CACHE[ns_key] = _build(list(ns_key))
    return nc


def _host_prep(x, lengths, W1, b1, W2, b2, Wl1, bl1, Wl2, bl2, Wl3, bl3):
    bf16 = ml_dtypes.bfloat16
    x = np.asarray(x, np.float32)
    lengths = np.asarray(lengths, np.int32)
    W1 = np.asarray(W1, np.float32)      # [D,1,3,D]
    b1 = np.asarray(b1, np.float32)
    W2 = np.asarray(W2, np.float32).reshape(D, D)
    b2 = np.asarray(b2, np.float32)
    Wl1 = np.asarray(Wl1, np.float32)    # [256, D]
    bl1 = np.asarray(bl1, np.float32)
    Wl2 = np.asarray(Wl2, np.float32)
    bl2 = np.asarray(bl2, np.float32)
    Wl3 = np.asarray(Wl3, np.float32)
    bl3 = np.asarray(bl3, np.float32)

    # ---- sort by length, assign ranks: slot s <- ranks [16s, 16s+16),
    # core c gets ranks 16s+2c (pos 0 -> partitions 0:64) and 16s+2c+1.
    order = np.argsort(-lengths, kind="stable")
    Ls = lengths[order]
    ns_list = []
    for s in range(NSLOT):
        mx = int(Ls[16 * s:16 * s + 16].max())
        ns_list.append(max(1, -(-mx // TC)))
    ns_key = tuple(ns_list)

    wk = W1[:, 0]                        # [F, 3, D]; h += x[t+k] @ wk[:,k].T

    # ---- per-item device frames + exact host-side pooling correction
    # S_dev(item) = sum_{t<512n_s-2} relu(h~[t]) with x zero-padded at L.
    # corr = S_dev - S_true, pre-divided by (L-2).
    relu_b1 = np.maximum(b1, 0.0)        # [D]
    corr_sorted = np.zeros((B, D), np.float32)
    for r in range(B):
        it = order[r]
        L = int(lengths[it])
        M = TC * ns_list[r // 16]        # frames loaded for this item
        c = np.zeros(D, np.float32)
        c += max(0, M - 2 - L) * relu_b1
        if L - 2 <= M - 3:
            c += np.maximum(b1 + wk[:, 0] @ x[it, L - 2] + wk[:, 1] @ x[it, L - 1], 0.0)
        if L - 1 <= M - 3:
            c += np.maximum(b1 + wk[:, 0] @ x[it, L - 1], 0.0)
        corr_sorted[r] = c / (L - 2)

    # ---- shared (weight) inputs, host-transposed/packed
    wpk = np.zeros((128, 3 * 128), np.float32)
    for k in range(3):
        wkT = wk[:, k].T                 # [D(in), F(out)]
        wpk[0:64, 128 * k:128 * k + 64] = wkT
        wpk[64:128, 128 * k + 64:128 * (k + 1)] = wkT
    bpk = np.concatenate([b1, b1]).reshape(128, 1)

    # fold td2 into layer 1:  z1 = relu(Wl1 @ (W2 p + b2) + bl1)
    Wl1f = Wl1 @ W2                      # [256, 64]
    bl1f = Wl1 @ b2 + bl1                # [256]
    wl1 = np.ascontiguousarray(Wl1f.T)   # [64, 256]
    wl2 = np.ascontiguousarray(Wl2.T)    # [256, 256] -> [128, 2*256]
    wl2 = wl2.reshape(2, 128, 256).transpose(1, 0, 2).reshape(128, 512)
    wl3 = np.ascontiguousarray(Wl3.T)    # [256, 512] -> [128, 2*512]
    wl3 = wl3.reshape(2, 128, 512).transpose(1, 0, 2).reshape(128, 1024)
    fp8 = ml_dtypes.float8_e4m3fn
    shared = {
        "wpk": wpk.astype(fp8),
        "bpk": bpk,
        "wl1": wl1.astype(bf16),
        "wl2": np.ascontiguousarray(wl2).astype(bf16),
        "wl3": np.ascontiguousarray(wl3).astype(bf16),
        "bl1": np.ascontiguousarray(bl1f.reshape(2, 128).T),
        "bl2": np.ascontiguousarray(bl2.reshape(2, 128).T),
        "bl3": np.ascontiguousarray(bl3.reshape(4, 128).T),
    }

    # ---- per-core ragged transposed bf16 x (slot-contiguous), inv, corr
    in_maps = []
    for c in range(NCORES):
        inv = np.zeros((128, NSLOT), np.float32)
        corr = np.zeros((64, BS), np.float32)
        m = {"inv": inv, "corr": corr, **shared}
        for s in range(NSLOT):
            F_s = TC * ns_list[s]
            xt = np.zeros((128, F_s + 2), fp8)
            for pos in range(2):
                r = 16 * s + 2 * c + pos
                it = order[r]
                L = int(lengths[it])
                n_use = min(L, F_s)
                xt[64 * pos:64 * pos + 64, :n_use] = \
                    x[it, :n_use].T.astype(fp8)
                inv[64 * pos:64 * pos + 64, s] = 1.0 / (L - 2)
                corr[:, 2 * s + pos] = corr_sorted[r]
            m[f"xt{s}"] = xt
        in_maps.append(m)

    return ns_key, in_maps, order


def _gather_out(per_core_y, order):
    # y_sb[:, m*BS + j] = feats[m*128:(m+1)*128] of device item j = 2s+pos
    # on core c  -> global rank 16s+2c+pos.
    out = np.empty((B, OUT), np.float32)
    for c in range(NCORES):
        Y = np.asarray(per_core_y[c], np.float32)   # [128, 4*BS]
        feats = Y.reshape(128, 4, BS).transpose(2, 1, 0).reshape(BS, OUT)
        for j in range(BS):
            s, pos = divmod(j, 2)
            out[order[16 * s + 2 * c + pos]] = feats[j]
    return out


def kernel(x, lengths, W1, b1, W2, b2, Wl1, bl1, Wl2, bl2, Wl3, bl3,
           _want_trace=False, **_ignored):
    from concourse.bass_utils import run_bass_kernel_spmd

    ns_key, in_maps, order = _host_prep(
        x, lengths, W1, b1, W2, b2, Wl1, bl1, Wl2, bl2, Wl3, bl3)
    nc = _get_nc(ns_key)
    res = run_bass_kernel_spmd(nc, in_maps, list(range(NCORES)),
                               trace=_want_trace)
    if _want_trace:
        _CACHE["last_result"] = res
    return _gather_out([res.results[c]["y"] for c in range(NCORES)], order)
